# revision 13
# baseline (speedup 1.0000x reference)
"""Trainium2 Bass kernel for nn_CrossAttention (sampling, memory-bound).

Computation (per the reference):
  item_q = item_query @ W.T + b ; user_q = user_query @ W.T + b
  energy_u[n,k] = <item_q[n], user_context[n,k]>  (masked, softmax over k)
  energy_i[n,k] = <user_q[n], item_context[n,k]>  (masked, softmax over k)
  uw = one_hot(argmax(softmax_u + gumbel1)) ; iw = one_hot(argmax(softmax_i + gumbel2))
  predicted = sigmoid(user_context[n, idx_u] * item_context[n, idx_i])
(the tanh attention outputs of the reference are dead code — never used in outputs)

Sharding: pure data-parallel over batch N=1024 -> 128 samples/core on 8 cores.

Per-core device strategy (all f32, no transposes needed):
  - context tiles streamed in natural [n=128 part, k-block, e=512] layout (16KB
    contiguous per partition per DMA)
  - energy dot products via VectorE fused tensor_tensor_reduce (mult+add-reduce
    in one pass over the free dim) -> ~690ns per (k) per tensor, 400 total
  - softmax via reduce_max + ScalarE Exp(bias=-max, accum_out=Z) + reciprocal
  - gumbel argmax via VectorE max/max_index over p + g (g = fixed threefry noise
    from key 42, recomputed bit-exactly on host CPU)
  - one-hot via is_equal against an iota; final rows gathered with an
    indirect (gather) DMA; sigmoid on ScalarE.
"""

import numpy as np

import concourse.bacc as bacc
import concourse.bass as bass
import concourse.mybir as mybir
import concourse.tile as tile

N_FULL = 1024
N_CORES = 8
NLOC = N_FULL // N_CORES  # 128
K = 200
E = 512
E_IN = 768
E_AUG = 896  # 768 + 1 (bias row), zero-padded to 7*128
N_CHUNK = E_AUG // 128  # 7
KB = 8  # k's per context DMA block
NKB = K // KB  # 25
F32 = mybir.dt.float32
NEG_BIG = -1.0e20


def build_nc(reps=1):
    # Bacc (not raw Bass): its compile() runs generate_event_semaphores, which
    # splits multi-wait instructions to satisfy the TRN2 1-wait-per-instruction
    # constraint, and lowers the sem range-clear walrus can't encode.
    # reps>1 replicates the steady-state body (for timing measurements only).
    nc = bacc.Bacc(trn_type="TRN2")

    def din(name, shape):
        return nc.dram_tensor(name, shape, F32, kind="ExternalInput").ap()

    uctx = din("user_ctx", [NLOC, K, E])
    ictx = din("item_ctx", [NLOC, K, E])
    # packed [896, 512 (W.T;b) | 128 (item_q.T;1) | 128 (user_q.T;1)] so the
    # transform matmuls depend on a single DMA completion (walrus allows only
    # a limited number of sync waits per Matmult instruction)
    wq_pack = din("wq_pack", [E_AUG, E + 2 * NLOC])
    msk = {"u": din("msk_u", [NLOC, K]), "i": din("msk_i", [NLOC, K])}
    gum = {"u": din("gum_u", [NLOC, K]), "i": din("gum_i", [NLOC, K])}

    pred = nc.dram_tensor("pred", [NLOC, E], F32, kind="ExternalOutput").ap()
    onehot_out = {
        "u": nc.dram_tensor("uw", [NLOC, K], F32, kind="ExternalOutput").ap(),
        "i": nc.dram_tensor("iw", [NLOC, K], F32, kind="ExternalOutput").ap(),
    }

    Alu = mybir.AluOpType
    Act = mybir.ActivationFunctionType

    with tile.TileContext(nc) as tc:
        with (
            tc.tile_pool(name="const", bufs=1) as constp,
            tc.tile_pool(name="ctx", bufs=3) as ctxp,
            tc.tile_pool(name="work", bufs=1) as workp,
            tc.tile_pool(name="psum", bufs=2, space="PSUM") as psump,
        ):
            # ---- constants (ACT HWDGE ring; SP ring stays free for ctx stream) ----
            WQ = E + 2 * NLOC
            wq_sb = constp.tile([128, N_CHUNK, WQ], F32, tag="wq_sb")
            nc.scalar.dma_start(
                out=wq_sb, in_=wq_pack.rearrange("(c p) m -> p c m", p=128)
            )
            w_sb = wq_sb[:, :, 0:E]
            qt_sb = {
                "item": wq_sb[:, :, E : E + NLOC],
                "user": wq_sb[:, :, E + NLOC : E + 2 * NLOC],
            }

            gum_sb = {}
            msk_sb = {}
            for nm in ("u", "i"):
                g = constp.tile([NLOC, K], F32, tag=f"gum_{nm}")
                nc.scalar.dma_start(out=g, in_=gum[nm])
                gum_sb[nm] = g
                m = constp.tile([NLOC, K], F32, tag=f"msk_{nm}")
                nc.scalar.dma_start(out=m, in_=msk[nm])
                msk_sb[nm] = m

            iota_k_i = constp.tile([NLOC, K], mybir.dt.int32, tag="iota_k_i")
            nc.gpsimd.iota(iota_k_i, pattern=[[1, K]], base=0, channel_multiplier=0)
            iota_k_f = constp.tile([NLOC, K], F32, tag="iota_k_f")
            nc.vector.tensor_copy(out=iota_k_f, in_=iota_k_i)
            # per-partition base row index n*K for the gather
            iota_row = constp.tile([NLOC, 1], mybir.dt.uint32, tag="iota_row")
            nc.gpsimd.iota(iota_row, pattern=[[0, 1]], base=0, channel_multiplier=K)

            # ---- q transforms (q = [query,1,pad] @ [W.T;b;pad]) on TensorE ----
            # item query attends user context; user query attends item context
            q_sb = {}
            for nm, qt in (("u", qt_sb["item"]), ("i", qt_sb["user"])):
                ps = psump.tile([128, E], F32, tag="q_psum")
                for ci in range(N_CHUNK):
                    nc.tensor.matmul(
                        ps,
                        lhsT=qt[:, ci, :],
                        rhs=w_sb[:, ci, :],
                        start=(ci == 0),
                        stop=(ci == N_CHUNK - 1),
                    )
                t = constp.tile([NLOC, E], F32, tag=f"q_{nm}")
                nc.scalar.copy(out=t, in_=ps)
                q_sb[nm] = t

            # ---- energies + sampling per context tensor ----
            scratch = workp.tile([NLOC, E], F32, tag="ttr_scratch")
            for _rep in range(reps):
              gathered = {}
              for nm, ctx_dr in (("u", uctx), ("i", ictx)):
                q = q_sb[nm]
                energy = workp.tile([NLOC, K], F32, tag=f"energy_{nm}")
                for kb in range(NKB):
                    ct = ctxp.tile([NLOC, KB, E], F32, tag="ctxtile")
                    nc.sync.dma_start(out=ct, in_=ctx_dr[:, kb * KB : (kb + 1) * KB, :])
                    for j in range(KB):
                        k = kb * KB + j
                        # fused dot: out = (ct*1.0)*q elementwise, accum = sum
                        # (tensor_tensor_reduce faults on this device's ucode;
                        # scalar_tensor_tensor with accum_out is equivalent)
                        nc.vector.scalar_tensor_tensor(
                            out=scratch,
                            in0=ct[:, j, :],
                            scalar=1.0,
                            in1=q,
                            op0=Alu.mult,
                            op1=Alu.mult,
                            accum_out=energy[:, k : k + 1],
                        )
                # mask: energy += (-1e20 * mask)  (bit-identical to where())
                nc.vector.tensor_add(out=energy, in0=energy, in1=msk_sb[nm])
                mx = workp.tile([NLOC, 1], F32, tag=f"mx_{nm}")
                nc.vector.reduce_max(out=mx, in_=energy, axis=mybir.AxisListType.X)
                negm = workp.tile([NLOC, 1], F32, tag=f"negm_{nm}")
                nc.scalar.mul(out=negm, in_=mx, mul=-1.0)
                ex = workp.tile([NLOC, K], F32, tag=f"ex_{nm}")
                zs = workp.tile([NLOC, 1], F32, tag=f"z_{nm}")
                nc.scalar.activation(
                    out=ex, in_=energy, func=Act.Exp, bias=negm, scale=1.0, accum_out=zs
                )
                rz = workp.tile([NLOC, 1], F32, tag=f"rz_{nm}")
                nc.vector.reciprocal(out=rz, in_=zs)
                # y = p + g = ex * (1/Z) + gumbel
                y = workp.tile([NLOC, K], F32, tag=f"y_{nm}")
                nc.vector.scalar_tensor_tensor(
                    out=y,
                    in0=ex,
                    scalar=rz[:, 0:1],
                    in1=gum_sb[nm],
                    op0=Alu.mult,
                    op1=Alu.add,
                )
                mx8 = workp.tile([NLOC, 8], F32, tag=f"mx8_{nm}")
                nc.vector.max(out=mx8, in_=y)
                idx8 = workp.tile([NLOC, 8], mybir.dt.uint32, tag=f"idx8_{nm}")
                nc.vector.max_index(out=idx8, in_max=mx8, in_values=y)
                idxf = workp.tile([NLOC, 1], F32, tag=f"idxf_{nm}")
                nc.vector.tensor_copy(out=idxf, in_=idx8[:, 0:1])
                oh = workp.tile([NLOC, K], F32, tag=f"oh_{nm}")
                nc.vector.tensor_scalar(
                    out=oh,
                    in0=iota_k_f,
                    scalar1=idxf[:, 0:1],
                    scalar2=None,
                    op0=Alu.is_equal,
                )
                nc.scalar.dma_start(out=onehot_out[nm], in_=oh)
                # gather ctx[n, idx[n], :] via indirect DMA on rows of [(n k), e]
                rowi = workp.tile([NLOC, 1], mybir.dt.uint32, tag=f"row_{nm}")
                nc.vector.tensor_add(out=rowi, in0=iota_row, in1=idx8[:, 0:1])
                gt = workp.tile([NLOC, E], F32, tag=f"gath_{nm}")
                nc.gpsimd.indirect_dma_start(
                    out=gt,
                    out_offset=None,
                    in_=ctx_dr.rearrange("n k e -> (n k) e"),
                    in_offset=bass.IndirectOffsetOnAxis(ap=rowi[:, 0:1], axis=0),
                )
                gathered[nm] = gt

              # ---- predicted = sigmoid(gathered_u * gathered_i) ----
              prod = workp.tile([NLOC, E], F32, tag="prod")
              nc.vector.tensor_mul(out=prod, in0=gathered["u"], in1=gathered["i"])
              sg = workp.tile([NLOC, E], F32, tag="sg")
              nc.scalar.activation(out=sg, in_=prod, func=Act.Sigmoid)
              nc.scalar.dma_start(out=pred, in_=sg)

    nc.compile()
    return nc


def _gumbel_noise():
    """Recompute the reference's fixed gumbel noise (key 42) bit-exactly on CPU."""
    import jax
    import jax.numpy as jnp

    cpu = jax.devices("cpu")[0]
    with jax.default_device(cpu):
        gk1, gk2 = jax.random.split(jax.random.key(42))
        u1 = jax.random.uniform(gk1, (N_FULL, K), minval=1e-10, maxval=1.0)
        u2 = jax.random.uniform(gk2, (N_FULL, K), minval=1e-10, maxval=1.0)
        g1 = -jnp.log(-jnp.log(u1))
        g2 = -jnp.log(-jnp.log(u2))
        return np.asarray(g1, np.float32), np.asarray(g2, np.float32)


def make_in_maps(
    user_query, item_query, user_context, item_context, user_key_mask, item_key_mask, W, b
):
    user_query = np.ascontiguousarray(np.asarray(user_query, np.float32))
    item_query = np.ascontiguousarray(np.asarray(item_query, np.float32))
    user_context = np.ascontiguousarray(np.asarray(user_context, np.float32))
    item_context = np.ascontiguousarray(np.asarray(item_context, np.float32))
    W = np.asarray(W, np.float32)
    b = np.asarray(b, np.float32)
    msk_u = np.where(np.asarray(user_key_mask), np.float32(NEG_BIG), np.float32(0.0))
    msk_i = np.where(np.asarray(item_key_mask), np.float32(NEG_BIG), np.float32(0.0))
    msk_u = np.ascontiguousarray(msk_u.astype(np.float32))
    msk_i = np.ascontiguousarray(msk_i.astype(np.float32))

    w_aug = np.zeros((E_AUG, E), np.float32)
    w_aug[:E_IN, :] = W.T
    w_aug[E_IN, :] = b

    def qt_aug(q):
        t = np.zeros((E_AUG, N_FULL), np.float32)
        t[:E_IN, :] = q.T
        t[E_IN, :] = 1.0
        return t

    qtu_full = qt_aug(user_query)
    qti_full = qt_aug(item_query)

    g1, g2 = _gumbel_noise()

    in_maps = []
    for c in range(N_CORES):
        sl = slice(c * NLOC, (c + 1) * NLOC)
        wq = np.concatenate(
            [w_aug, qti_full[:, sl], qtu_full[:, sl]], axis=1
        )  # [896, 512+128+128]
        in_maps.append(
            {
                "user_ctx": user_context[sl],
                "item_ctx": item_context[sl],
                "wq_pack": np.ascontiguousarray(wq),
                "msk_u": msk_u[sl],
                "msk_i": msk_i[sl],
                "gum_u": np.ascontiguousarray(g1[sl]),
                "gum_i": np.ascontiguousarray(g2[sl]),
            }
        )
    return in_maps


_RESULT_CACHE = {}


def kernel(
    user_query,
    item_query,
    user_context,
    item_context,
    user_key_mask,
    item_key_mask,
    W,
    b,
    _trace=False,
):
    from concourse.bass_utils import run_bass_kernel_spmd

    in_maps = make_in_maps(
        user_query, item_query, user_context, item_context,
        user_key_mask, item_key_mask, W, b,
    )
    nc = build_nc()
    res = run_bass_kernel_spmd(
        nc, in_maps, core_ids=list(range(N_CORES)), trace=_trace
    )
    _RESULT_CACHE["last"] = res

    pred = np.concatenate([r["pred"] for r in res.results], axis=0)
    uw = np.concatenate([r["uw"] for r in res.results], axis=0)
    iw = np.concatenate([r["iw"] for r in res.results], axis=0)
    return pred[:, None, :], uw[:, None, :], iw[:, None, :]
